# revision 1
# baseline (speedup 1.0000x reference)
"""Trainium2 Bass kernel for a dense transformer block (B=2, T=2048, C=1024, H=16).

Sharding over 8 NeuronCores (same topology as the f32r baseline):
  - LN / QKV / proj / MLP row-sharded: core c owns 512 contiguous token rows
    (batch c//4, rows [512*(c%4), 512*(c%4+1))).
  - Attention head-sharded: core d owns heads {2d, 2d+1} for BOTH batches
    (4 (batch, head) pairs per core); three AllToAlls re-shard q/k, v, and y.

Precision strategy (rel-err budget 2e-2; attention contributes ~2% of the
output magnitude, MLP ~35%):
  - Attention path entirely fp8: q/k/v/es/h in e4m3, y/w_proj in e5m2.
    All attention-side matmuls use the fp8 DoubleRow perf mode (2 k-tiles
    per instruction at 0.5 cycles/row).
  - MLP (fc/cproj) in bf16 -- fp8 there would cost ~2e-2 alone.
  - Causal mask is accumulated into the score psum by an fp8 matmul
    (identity-stationary, mask moving; -448*2^-5 = -14 -> exp ~ 8e-7),
    so every strip takes exactly one Exp and no mask multiply.
  - ln weights are folded into w_attn / w_fc on the host (exact identity).
  - The MLP intermediate lives in SBUF (bf16, 32 KB/partition) -- no DRAM
    roundtrip.

Collectives (fp8/fp8e5) shrink to 1 MB + 0.5 MB + 0.5 MB and overlap:
a2a#1(qk) runs over the v-matmul tail, a2a#2(v) under the score/exp stream,
and MLP weights prefetch under attention.
"""

from contextlib import ExitStack

import numpy as np

import concourse.bacc as bacc
import concourse.bass as bass
import concourse.mybir as mybir
import concourse.tile as tile
from concourse.bass_utils import run_bass_kernel_spmd

P = 128
B, T, C, H, Dh = 2, 2048, 1024, 16, 64
NCORES = 8
R = 512          # token rows per core
RT = R // P      # 4 row tiles
CK = C // P      # 8 C-chunks
F32 = mybir.dt.float32
BF16 = mybir.dt.bfloat16
F8 = mybir.dt.float8e4
F85 = mybir.dt.float8e5
DR = mybir.MatmulPerfMode.DoubleRow
EPS = 1e-5
SCALE = float(C) ** -0.5  # 2**-5
# exp(u) ~ QA*(u+QB)**2 + QC on |u|<=0.8 (max rel err 2.1%) for the DVE
# side-channel that offloads part of the Act-bound exp stream
QA, QB, QC = 0.4802831261515508, 1.1196791965149733, 0.40954772142747264
QSA = QA ** 0.5
MASKV = -240.0            # e4m3 (IEEE) max finite; exp(-240/32)=5.5e-4 -> fp8 rounds to exactly 0

# a1q per-core buffer: [8 dests][qk 2][128 part][512 rows] fp8
QKP = P * R               # 65536 B per (dest, q/k)
# a1v per-core buffer: [8 dests][128 part][hh 2][cspair 2][cs2 2][64] fp8
VP = P * 512
# a2 per-core buffer: [8 dests][128 dims][512 rows] fp8e5

_CACHE = {}


def _ln_tile(nc, sm, dst, src, eps_t):
    """dst = layer_norm(src) (gain folded into the next weights)."""
    stats = sm.tile([P, 2, 6], F32, tag="stats", bufs=2)
    for g in range(2):
        nc.vector.bn_stats(out=stats[:, g, :], in_=src[:, g * 512:(g + 1) * 512])
    mv = sm.tile([P, 2], F32, tag="mv", bufs=2)
    nc.vector.bn_aggr(out=mv[:], in_=stats[:])
    rstd = sm.tile([P, 1], F32, tag="rstd", bufs=2)
    nc.scalar.activation(
        out=rstd[:], in_=mv[:, 1:2], func=mybir.ActivationFunctionType.Sqrt,
        bias=eps_t[:], scale=1.0,
    )
    nc.vector.reciprocal(out=rstd[:], in_=rstd[:])
    nc.vector.tensor_scalar(
        out=dst, in0=src, scalar1=mv[:, 0:1], scalar2=rstd[:],
        op0=mybir.AluOpType.subtract, op1=mybir.AluOpType.mult,
    )


def build():
    nc = bacc.Bacc(None, target_bir_lowering=False)

    # host-pretiled inputs (see _make_in_maps for layouts)
    xin = nc.declare_dram_parameter("xin", [P, RT * C], F32, isOutput=False)
    wqk8 = nc.declare_dram_parameter("wqk8", [8, P, 2 * 4 * 2 * P], F8, isOutput=False)
    wv8 = nc.declare_dram_parameter("wv8", [2, P, 8, 512], F8, isOutput=False)
    wp8 = nc.declare_dram_parameter("wp8", [8, P, 2 * 512], F85, isOutput=False)
    wft = nc.declare_dram_parameter("wft", [16, P, 2 * C], BF16, isOutput=False)
    wct = nc.declare_dram_parameter("wct", [8, P, 4 * 2 * 512], BF16, isOutput=False)
    identb = nc.declare_dram_parameter("identb", [P, P], BF16, isOutput=False)
    idpair8 = nc.declare_dram_parameter("idpair8", [P, 2 * P], F8, isOutput=False)
    maskd = nc.declare_dram_parameter("maskd", [4, P, 2 * 512], F8, isOutput=False)
    out = nc.declare_dram_parameter("out", [R, C], F32, isOutput=True)

    with tile.TileContext(nc) as tc, ExitStack() as ctx:
        const = ctx.enter_context(tc.tile_pool(name="const", bufs=1))
        big = ctx.enter_context(tc.tile_pool(name="big", bufs=1))
        wcol = ctx.enter_context(tc.tile_pool(name="wcol", bufs=3))
        wrow = ctx.enter_context(tc.tile_pool(name="wrow", bufs=2))
        wctp = ctx.enter_context(tc.tile_pool(name="wctp", bufs=1))
        kv = ctx.enter_context(tc.tile_pool(name="kv", bufs=2))
        sm = ctx.enter_context(tc.tile_pool(name="sm", bufs=4))
        ev = ctx.enter_context(tc.tile_pool(name="ev", bufs=3))
        esp = ctx.enter_context(tc.tile_pool(name="esp", bufs=4))
        mtp = ctx.enter_context(tc.tile_pool(name="mtp", bufs=1))
        ps = ctx.enter_context(tc.tile_pool(name="ps", bufs=6, space="PSUM"))
        dram = ctx.enter_context(tc.tile_pool(name="dram", bufs=1, space="DRAM"))

        # ---------- constants ----------
        idtb = const.tile([P, P], BF16, name="idtb")
        nc.sync.dma_start(idtb[:], identb[:])
        idp8 = const.tile([P, 2, P], F8, name="idp8")
        nc.sync.dma_start(idp8[:].rearrange("p a b -> p (a b)"), idpair8[:])
        mDz = const.tile([P, 4, 2, 512], F8, name="mDz")
        nc.sync.dma_start(
            mDz[:].rearrange("p i a c -> p i (a c)"),
            maskd[:].rearrange("i p c -> p i c"),
        )
        eps_t = const.tile([P, 1], F32, name="eps_t")
        nc.any.memset(eps_t[:], EPS)

        # ---------- collective DRAM buffers ----------
        a1_in = dram.tile([NCORES, 3, P, R], F8, name="a1_in")
        a1_out = dram.tile([NCORES, 3, P, R], F8, name="a1_out")
        a2_in = dram.tile([NCORES, P, R], F85, name="a2_in")
        a2_out = dram.tile([NCORES, P, R], F85, name="a2_out")

        # ---------- phase 1: load x, LN1 -> h (bf16), transpose -> hT8 ----------
        with nc.named_scope("ln1"):
            xres = big.tile([P, RT, C], F32, tag="x", name="xres")
            h = big.tile([P, RT, C], BF16, tag="h", name="h")
            for m in range(RT):
                nc.sync.dma_start(xres[:, m, :], xin[:, m * C:(m + 1) * C])
                _ln_tile(nc, sm, h[:, m, :], xres[:, m, :], eps_t)
            hT8 = big.tile([P, CK, R], F8, tag="ht8", name="hT8")
            for k in range(CK):
                pt = ps.tile([P, 512], BF16, tag="ps")
                for m in range(RT):
                    nc.tensor.matmul(
                        pt[:, m * P:(m + 1) * P],
                        h[:, m, k * P:(k + 1) * P],
                        idtb[:],
                        is_transpose=True,
                        start=(m == 0), stop=(m == RT - 1),
                    )
                nc.scalar.copy(hT8[:, k, :], pt[:])

        # ---------- phase 2: q/k matmuls (fp8 DR) -> a2a#1 ----------
        with nc.named_scope("qk"):
            for mp in range(8):
                wb = wcol.tile([P, 2, 4, 2, P], F8, tag="wc", name="wb")
                nc.sync.dma_start(
                    wb[:].rearrange("p m i a q -> p (m i a q)"), wqk8[mp]
                )
                et2 = ev.tile([P, 2, 512], F8, tag="ev", name="et2")
                for m2 in range(2):
                    m = 2 * mp + m2
                    pm = ps.tile([P, 512], F32, tag="ps")
                    for kp in range(4):
                        nc.tensor.matmul(
                            pm[:], wb[:, m2, kp, :, :], hT8[:, 2 * kp:2 * kp + 2, :],
                            start=(kp == 0), stop=(kp == 3),
                            perf_mode=DR,
                        )
                    nc.vector.tensor_copy(et2[:, m2, :], pm[:])
                d = (2 * mp) % 8
                nc.sync.dma_start(
                    a1_in[d:d + 2, mp // 4].rearrange("a p c -> p a c"), et2[:]
                )

        # ---------- phase 3: V (natural layout, plain fp8) -> a2a#2 ----------
        with nc.named_scope("v"):
            for half in range(2):
                wv = wrow.tile([P, CK, 512], F8, tag="wr", name="wv")
                nc.sync.dma_start(
                    wv[:].rearrange("p k c -> p (k c)"), wv8[half]
                )
                vbuf = ev.tile([P, 4, 2, 4, Dh], F8, tag="vbuf", bufs=1, name="vbuf")
                for m in range(RT):
                    pv = ps.tile([P, 512], F32, tag="ps")
                    for k in range(CK):
                        nc.tensor.matmul(
                            pv[:], hT8[:, k, m * P:(m + 1) * P], wv[:, k, :],
                            start=(k == 0), stop=(k == CK - 1),
                        )
                    nc.vector.tensor_copy(
                        vbuf[:, :, :, m, :],
                        pv[:].rearrange("p (dq hh x) -> p dq hh x", dq=4, hh=2),
                    )
                vdst = a1_in[4 * half:4 * half + 4, 2].rearrange(
                    "d p (hh y) -> d p hh y", hh=2
                )
                for hh in range(2):
                    nc.sync.dma_start(
                        vdst[:, :, hh, :].rearrange("d p y -> p d y"),
                        vbuf[:, :, hh, :, :].rearrange("p dq m x -> p dq (m x)"),
                    )

        wcts = [
            wctp.tile([P, 4, 2, 512], BF16, tag=f"wct{i}", name=f"wct{i}")
            for i in range(8)
        ]
        wfb = [
            wcol.tile([P, 2 * C], BF16, tag="wfpre", bufs=2, name=f"wf{i}")
            for i in range(2)
        ]
        nc.gpsimd.collective_compute(
            "AllToAll",
            mybir.AluOpType.bypass,
            ins=[a1_in[:].opt()],
            outs=[a1_out[:].opt()],
            replica_groups=[list(range(NCORES))],
        )

        # prefetch MLP weights under the collective: the time floor keeps
        # these 9.5 MB off the DMA engines until the a1_in writes are done
        with tc.tile_wait_until(0.042):
            for i in range(8):
                nc.scalar.dma_start(
                    wcts[i][:].rearrange("p i h c -> p (i h c)"), wct[i])
            for i in range(2):
                nc.scalar.dma_start(wfb[i][:], wft[i])

        # ---------- phase 4: attention (4 (batch, head) pairs per core) ----
        qcnt = [0]
        with nc.named_scope("attn"):
            for p_i in range(4):
                b = p_i // 2
                hl = p_i % 2
                sb = 4 * b
                # kt: [32, strip 16, half 2, 128] from K regions of 4 shards
                kt = kv.tile([32, 16, 2, P], F8, tag="kt", bufs=1, name="kt")
                # qt: [32, jq 4, half 2, 512] from Q regions
                qt = kv.tile([32, 4, 2, R], F8, tag="qt", bufs=1, name="qt")
                for s in range(4):
                    ksrc = a1_out[sb + s, 1].rearrange(
                        "(hh half p) r -> hh half p r", hh=2, half=2
                    )[hl]
                    nc.sync.dma_start(
                        kt[:, 4 * s:4 * s + 4, :, :],
                        ksrc.rearrange("half p (cs r2) -> p cs half r2", cs=4),
                    )
                    qsrc = a1_out[sb + s, 0].rearrange(
                        "(hh half p) r -> hh half p r", hh=2, half=2
                    )[hl]
                    nc.sync.dma_start(
                        qt[:, s, :, :], qsrc.rearrange("half p r -> p half r"),
                    )
                # vv: [128, shard 4, cspair 2, {cs2 2, 64 dims | ones}]
                # loaded on the DVE queue: it waits on a2a#2, and must not
                # stall the SP queue (weight prefetches, next pair's kt/qt)
                # col 0 = ones (denominator lands on psum partition 0),
                # cols 1-64 = v dims, cols 65-127 = zero padding
                vv = kv.tile([P, 4, 2, 2, P], F8, tag="vv", bufs=1, name="vv")
                nc.gpsimd.memset(vv[:].rearrange("p s a b x -> p (s a b x)"), 0.0)
                nc.gpsimd.memset(vv[:, :, :, :, 0:1], 1.0)
                for s in range(4):
                    vsrc = a1_out[sb + s, 2].rearrange(
                        "p (hh a b x) -> p hh a b x", hh=2, a=2, b=2)
                    nc.sync.dma_start(vv[:, s, :, :, 1:Dh + 1], vsrc[:, hl])


                for jq in range(4):
                    py = ps.tile([P, 512], F32, tag="av", bufs=2)
                    ngp = 2 * jq + 2
                    for gp in range(ngp):
                        es2 = esp.tile([P, 2, 512], F8, tag="es", name="es2")
                        for g2 in range(2):
                            g = 2 * gp + g2
                            pS = ps.tile([P, 512], F32, tag="ps")
                            masked = g >= 4 * jq
                            if masked:
                                nc.tensor.matmul(
                                    pS[:], idp8[:], mDz[:, g - 4 * jq, :, :],
                                    start=True, stop=False, perf_mode=DR,
                                )
                            nc.tensor.matmul(
                                pS[:],
                                kt[:, g, :, :],
                                qt[:, jq, :, :],
                                start=not masked, stop=True, perf_mode=DR,
                            )
                            offload = False
                            if not masked:
                                offload = qcnt[0] % 12 < 5
                                qcnt[0] += 1
                            if offload:
                                # exp via DVE quadratic to unload the Act engine
                                qt1 = sm.tile([P, 512], BF16, tag="qt1", bufs=2)
                                nc.vector.tensor_scalar(
                                    out=qt1[:], in0=pS[:],
                                    scalar1=SCALE * QSA, scalar2=QB * QSA,
                                    op0=mybir.AluOpType.mult,
                                    op1=mybir.AluOpType.add,
                                )
                                qt2 = sm.tile([P, 512], BF16, tag="qt2", bufs=2)
                                nc.vector.tensor_mul(qt2[:], qt1[:], qt1[:])
                                nc.vector.tensor_scalar(
                                    out=es2[:, g2, :], in0=qt2[:],
                                    scalar1=QC, scalar2=1.0,
                                    op0=mybir.AluOpType.add,
                                    op1=mybir.AluOpType.mult,
                                )
                            else:
                                nc.scalar.activation(
                                    out=es2[:, g2, :], in_=pS[:],
                                    func=mybir.ActivationFunctionType.Exp,
                                    scale=SCALE,
                                )
                        nc.tensor.matmul(
                            py[:], vv[:, gp // 2, gp % 2, :, :], es2[:],
                            start=(gp == 0), stop=(gp == ngp - 1),
                            perf_mode=DR,
                        )
                    # normalize: y[d, q] / denom[q]; denom = psum row 0
                    rr0 = sm.tile([1, 512], F32, tag="rr", bufs=2)
                    nc.vector.reciprocal(out=rr0[:], in_=py[0:1, :])
                    bb = sm.tile([Dh + 1, 512], F32, tag="bb", bufs=1)
                    nc.gpsimd.partition_broadcast(bb[:], rr0[:], channels=Dh + 1)
                    yst = sm.tile([Dh + 1, 512], F85, tag="yst", bufs=2)
                    nc.vector.tensor_mul(yst[:], py[0:Dh + 1, :], bb[:])
                    d = 4 * b + jq
                    nc.gpsimd.dma_start(
                        a2_in[d, Dh * hl:Dh * (hl + 1), :], yst[1:Dh + 1, :]
                    )

        # ---------- phase 5: AllToAll #3 (yT back to row owners) ----------
        nc.gpsimd.collective_compute(
            "AllToAll",
            mybir.AluOpType.bypass,
            ins=[a2_in[:].opt()],
            outs=[a2_out[:].opt()],
            replica_groups=[list(range(NCORES))],
        )

        # ---------- phase 6: proj (fp8e5 DR) + residual into xres ----------
        with nc.named_scope("proj"):
            # yTm8: [p, m 4, kpair 4, k2 2, r2 128] fp8e5
            yTm8 = big.tile([P, RT, 4, 2, P], F85, tag="yt", name="yTm8")
            for s in range(NCORES):
                nc.sync.dma_start(
                    yTm8[:, :, s // 2, s % 2, :],
                    a2_out[s].rearrange("p (m r2) -> p m r2", m=RT),
                )
            wpb = wrow.tile([P, 4, 2, 2, 512], F85, tag="wp", bufs=1, name="wpb")
            nc.sync.dma_start(
                wpb[:].rearrange("p i h a c -> p (i h) (a c)"),
                wp8[:].rearrange("i p c -> p i c"),
            )
            for m in range(RT):
                for half in range(2):
                    pp = ps.tile([P, 512], F32, tag="ps")
                    for kp in range(4):
                        nc.tensor.matmul(
                            pp[:], yTm8[:, m, kp, :, :], wpb[:, kp, half, :, :],
                            start=(kp == 0), stop=(kp == 3), perf_mode=DR,
                        )
                    nc.vector.tensor_add(
                        xres[:, m, 512 * half:512 * half + 512], pp[:],
                        xres[:, m, 512 * half:512 * half + 512],
                    )

        # ---------- phase 7: LN2 + transpose (bf16) ----------
        with nc.named_scope("ln2"):
            h2 = big.tile([P, RT, C], BF16, tag="h", name="h2")
            for m in range(RT):
                _ln_tile(nc, sm, h2[:, m, :], xres[:, m, :], eps_t)
            h2T = big.tile([P, CK, R], BF16, tag="h2t", name="h2T")
            for k in range(CK):
                pt = ps.tile([P, 512], BF16, tag="ps")
                for m in range(RT):
                    nc.tensor.matmul(
                        pt[:, m * P:(m + 1) * P],
                        h2[:, m, k * P:(k + 1) * P],
                        idtb[:],
                        is_transpose=True,
                        start=(m == 0), stop=(m == RT - 1),
                    )
                nc.vector.tensor_copy(h2T[:, k, :], pt[:])

        # ---------- phase 8: fc (bf16) + relu -> mt (SBUF-resident) -------
        with nc.named_scope("mlp"):
            mts = []
            for m in range(32):
                if m < 4:
                    wb = wfb[m // 2]
                elif m % 2 == 0:
                    wb = wcol.tile([P, 2 * C], BF16, tag="wfpre", bufs=2, name="wbf")
                    nc.sync.dma_start(wb[:], wft[m // 2])
                wbv = wb[:].rearrange("p (i k q) -> p i k q", i=2, k=CK)
                pm = ps.tile([P, 512], F32, tag="ps")
                for k in range(CK):
                    nc.tensor.matmul(
                        pm[:], wbv[:, m % 2, k, :], h2T[:, k, :],
                        start=(k == 0), stop=(k == CK - 1),
                    )
                mtb = mtp.tile([P, 512], BF16, tag=f"mt{m}", name=f"mt{m}")
                nc.scalar.activation(
                    out=mtb[:], in_=pm[:],
                    func=mybir.ActivationFunctionType.Relu,
                )
                mts.append(mtb)

            # ---------- phase 9: cproj (bf16) + residual -> out ----------
            # weights already resident in wcts; one psum group per (m, half)
            out_r = out[:].rearrange("(m p) c -> p m c", p=P)
            for m in range(RT):
                for half in range(2):
                    pc = ps.tile([P, 512], F32, tag="ps")
                    for k in range(32):
                        nc.tensor.matmul(
                            pc[:],
                            mts[k][:, m * P:(m + 1) * P],
                            wcts[k // 4][:, k % 4, half, :],
                            start=(k == 0), stop=(k == 31),
                        )
                    ot = ev.tile([P, 512], F32, tag="ot", bufs=2, name="ot")
                    nc.vector.tensor_add(
                        ot[:], pc[:],
                        xres[:, m, 512 * half:512 * half + 512]
                    )
                    nc.sync.dma_start(
                        out_r[:, m, 512 * half:512 * half + 512], ot[:]
                    )

    nc.finalize()
    return nc


def _get_nc():
    if "nc" not in _CACHE:
        _CACHE["nc"] = build()
    return _CACHE["nc"]


def _make_in_maps(x, ln1_w, w_attn, w_proj, ln2_w, w_fc, w_cproj):
    import ml_dtypes
    E4 = ml_dtypes.float8_e4m3
    E5 = ml_dtypes.float8_e5m2
    BF = ml_dtypes.bfloat16

    x = np.asarray(x, dtype=np.float32)
    ln1_w = np.asarray(ln1_w, dtype=np.float32)
    ln2_w = np.asarray(ln2_w, dtype=np.float32)
    w_attn = np.asarray(w_attn, dtype=np.float32) * ln1_w[:, None]
    w_proj = np.asarray(w_proj, dtype=np.float32)
    w_fc = np.asarray(w_fc, dtype=np.float32) * ln2_w[:, None]
    w_cproj = np.asarray(w_cproj, dtype=np.float32)

    identb = np.eye(P, dtype=np.float32).astype(BF)
    idp = np.concatenate([np.eye(P, dtype=np.float32)] * 2, axis=1).astype(E4)
    # mask pairs: [i][p][slot 2][512]; slot0 = mask (0 / -448), slot1 = 0
    ii = np.arange(P)[:, None]
    jj = np.arange(512)[None, :]
    maskd = np.zeros((4, P, 2, 512), dtype=np.float32)
    for i in range(4):
        maskd[i, :, 0, :] = np.where(ii <= jj - P * i, 0.0, MASKV)
    maskd = maskd.reshape(4, P, 1024).astype(E4)

    # wqk8[m, kp, p, (k2 q)] = w_attn[128*(2kp+k2) + p, 128m + q]  (fp8 e4m3)
    wqk = w_attn[:, 0:2 * C]
    wqk8 = np.ascontiguousarray(
        wqk.reshape(4, 2, P, 16, P).transpose(3, 0, 2, 1, 4).reshape(16, 4, P, 2 * P)
        .reshape(8, 2, 4, P, 2 * P).transpose(0, 3, 1, 2, 4).reshape(8, P, 2 * 4 * 2 * P)
    ).astype(E4)
    # wv8[half, p, k, c] = w_attn[128k + p, 2048 + 512*half + c]
    wv_ = w_attn[:, 2 * C:3 * C]
    wv8 = np.ascontiguousarray(
        wv_.reshape(CK, P, 2, 512).transpose(2, 1, 0, 3)
    ).astype(E4)
    # wp8[(kp*2+half), p, (k2 c)] = w_proj[128*(2kp+k2)+p, 512half+c] (e5m2)
    wp8 = np.ascontiguousarray(
        w_proj.reshape(4, 2, P, 2, 512).transpose(0, 3, 2, 1, 4).reshape(8, P, 2 * 512)
    ).astype(E5)
    # wft[m, p, (i k q)]: i in {0,1} m-subtile, k = C chunk, q = 128 fc cols
    wft = np.ascontiguousarray(
        w_fc.reshape(CK, P, 32, P).transpose(2, 1, 0, 3).reshape(16, 2, P, CK * P)
        .transpose(0, 2, 1, 3).reshape(16, P, 2 * C)
    ).astype(BF)
    # wct[kg, p, (k4 half c)] = w_cproj[128*(4kg+k4)+p, 512*half+c]
    wct = np.ascontiguousarray(
        w_cproj.reshape(8, 4, P, 2, 512).transpose(0, 2, 1, 3, 4).reshape(8, P, 4 * 2 * 512)
    ).astype(BF)

    in_maps = []
    for c in range(NCORES):
        b = c // 4
        r0 = 512 * (c % 4)
        xr = x[b, r0:r0 + R]  # [512, 1024]
        xt = np.ascontiguousarray(
            xr.reshape(RT, P, C).transpose(1, 0, 2).reshape(P, RT * C)
        )
        in_maps.append({
            "xin": xt,
            "wqk8": wqk8, "wv8": wv8, "wp8": wp8, "wft": wft, "wct": wct,
            "identb": identb, "idpair8": idp, "maskd": maskd,
        })
    return in_maps


def run(x, ln1_w, w_attn, w_proj, ln2_w, w_fc, w_cproj, trace=False):
    nc = _get_nc()
    in_maps = _make_in_maps(x, ln1_w, w_attn, w_proj, ln2_w, w_fc, w_cproj)
    res = run_bass_kernel_spmd(nc, in_maps, list(range(NCORES)), trace=trace)
    out = np.empty((B, T, C), dtype=np.float32)
    for c in range(NCORES):
        b = c // 4
        r0 = 512 * (c % 4)
        out[b, r0:r0 + R] = res.results[c]["out"]
    return out, res


def kernel(x, ln1_w, w_attn, w_proj, ln2_w, w_fc, w_cproj):
    out, _ = run(x, ln1_w, w_attn, w_proj, ln2_w, w_fc, w_cproj)
    return out

